# revision 13
# baseline (speedup 1.0000x reference)
"""Trainium2 Bass kernel for the CompositionalCritic (nn_CompositionalCritic_18116172054929).

Math (per batch row b):
    x = concat(obs, act)                      # [160]
    h1 = relu(sum_k cw[k] * (x @ W1[k] + b1[k]))   # [1024]
    h2 = relu(sum_k cw[k] * (h1 @ W2[k] + b2[k]))  # [1024]
    out = h2 @ Wo + bo                        # [1]

Key transformation: the soft composition is linear, so
    sum_k cw[k] * (x @ W1[k]) = z @ W1_flat,   z[(k,i)] = cw[k] * x[i]
and the bias term sum_k cw[k]*b1[k] is 16 extra contraction rows with
activations = cw. Each layer becomes ONE dense matmul over an extended
contraction dim; no [B, K, H] intermediate is ever materialized.

Sharding: data-parallel over batch: 8 cores x 512 rows, weights replicated.
All matmuls run in float32r (fp32 storage, near-fp32 accuracy, bf16-class
PE throughput). Activations live feature-major [feat, b] on-chip so the
contraction dim is on partitions for every matmul.
"""

import numpy as np

import concourse.bass as bass
import concourse.mybir as mybir
import concourse.tile as tile
from concourse import bacc
from concourse.bass_utils import run_bass_kernel_spmd
from concourse.masks import make_identity

N_CORES = 8
B, OBS, ACT, K, H = 4096, 128, 32, 16, 1024
IN1 = OBS + ACT  # 160
BS = B // N_CORES  # 512 batch rows per core
NBT = BS // 128  # 4 batch tiles of 128
OT = H // 128  # 8 output tiles per layer
F32 = mybir.dt.float32
F32R = mybir.dt.float32r


def build_nc():
    nc = bacc.Bacc(
        "TRN2",
        target_bir_lowering=False,
        debug=False,
        enable_asserts=False,
        num_devices=N_CORES,
    )

    obs = nc.dram_tensor("obs", [BS, OBS], F32, kind="ExternalInput")
    act = nc.dram_tensor("actions", [BS, ACT], F32, kind="ExternalInput")
    cw = nc.dram_tensor("comp_weights", [BS, K], F32, kind="ExternalInput")
    W1 = nc.dram_tensor("W1", [K, IN1, H], F32R, kind="ExternalInput")
    b1 = nc.dram_tensor("b1", [K, H], F32R, kind="ExternalInput")
    W2 = nc.dram_tensor("W2", [K, H, H], F32R, kind="ExternalInput")
    b2 = nc.dram_tensor("b2", [K, H], F32R, kind="ExternalInput")
    Wo = nc.dram_tensor("Wo", [H, 1], F32R, kind="ExternalInput")
    bo = nc.dram_tensor("bo", [1, 1], F32, kind="ExternalInput")
    out = nc.dram_tensor("out", [1, BS], F32, kind="ExternalOutput")

    with tile.TileContext(nc) as tc:
        with (
            tc.tile_pool(name="persist", bufs=1) as persist,
            tc.tile_pool(name="ld", bufs=4) as ld,
            tc.tile_pool(name="w1p", bufs=4) as w1p,
            tc.tile_pool(name="w2p", bufs=6) as w2p,
            tc.tile_pool(name="zp", bufs=6) as zp,
            tc.tile_pool(name="cwrep", bufs=K) as cwrep,
            tc.tile_pool(name="ymaj", bufs=OT) as ymaj,
            tc.tile_pool(name="psum", bufs=8, space="PSUM") as psum,
        ):
            # ---- phase 0: transpose inputs to feature-major ----
            ident = persist.tile([128, 128], F32, tag="ident")
            make_identity(nc, ident)

            # cw_rep[k][p, b] = cwT[k, b] for all p: PE broadcast via the
            # expander E = kron(I_K, ones(1, 128)); cw_rep[k] = E[:, k].T @ cwT
            # expander[j, k, p] = (j == k): gpsimd affine_select, like make_identity
            expander = persist.tile([K, K, 128], F32, tag="expander")
            nc.gpsimd.memset(expander, 0.0)
            nc.gpsimd.affine_select(
                out=expander,
                in_=expander,
                compare_op=mybir.AluOpType.not_equal,
                fill=1.0,
                base=0,
                pattern=[[-1, K], [0, 128]],
                channel_multiplier=1,
            )
            exp_r = persist.tile([K, K, 128], F32R, tag="exp_r")
            nc.vector.tensor_copy(exp_r, expander)
            # exp2[j, g, ph, pl] = (j == 4g + ph): stacks 4 action-subtiles
            exp2 = persist.tile([K, 4, 4, 32], F32, tag="exp2")
            nc.gpsimd.memset(exp2, 0.0)
            nc.gpsimd.affine_select(
                out=exp2,
                in_=exp2,
                compare_op=mybir.AluOpType.not_equal,
                fill=1.0,
                base=0,
                pattern=[[-4, 4], [-1, 4], [0, 32]],
                channel_multiplier=1,
            )
            exp2_r = persist.tile([K, 4, 4, 32], F32R, tag="exp2_r")
            nc.vector.tensor_copy(exp2_r, exp2)


            xT0 = persist.tile([128, BS], F32R, tag="xT0")  # obsT
            xT1 = persist.tile([ACT, BS], F32R, tag="xT1")  # actionsT
            cwT = persist.tile([K, BS], F32R, tag="cwT")  # cw transposed

            for bt in range(NBT):  # cw first: it gates the broadcast chain
                bsl = bass.ts(bt, 128)
                cwb = ld.tile([128, K], F32, tag="cwb")
                nc.sync.dma_start(out=cwb, in_=cw[bsl, :])
                psc = psum.tile([K, 128], F32, tag="acc", name=f"tpc_{bt}")
                nc.tensor.transpose(psc[:, :], cwb[:, :], ident[:, :])
                nc.vector.tensor_copy(cwT[:, bsl], psc)
            for bt in range(NBT):
                bsl = bass.ts(bt, 128)
                ob = ld.tile([128, OBS], F32, tag="ob")
                nc.sync.dma_start(out=ob, in_=obs[bsl, :])
                pso = psum.tile([OBS, 128], F32, tag="acc", name=f"tpo_{bt}")
                nc.tensor.transpose(pso[:, :], ob[:, :], ident[:, :])
                nc.vector.tensor_copy(xT0[:, bsl], pso)

                ac = ld.tile([128, ACT], F32, tag="ac")
                nc.sync.dma_start(out=ac, in_=act[bsl, :])
                psa_t = psum.tile([ACT, 128], F32, tag="acc", name=f"tpa_{bt}")
                nc.tensor.transpose(psa_t[:, :], ac[:, :], ident[:, :])
                nc.vector.tensor_copy(xT1[:, bsl], psa_t)

            # replicate actionsT 4x vertically for the stacked L1 matmuls
            xT1r4 = persist.tile([128, BS], F32R, tag="xT1r4")
            for i in range(4):
                nc.sync.dma_start(out=xT1r4[bass.ts(i, ACT), :], in_=xT1[:, :])

            cw_rep = []
            cw_stack = []
            for k in range(K):
                pbc = psum.tile([128, BS], F32, tag="acc", name=f"bc_{k}")
                nc.tensor.matmul(
                    pbc[:, :], exp_r[:, k, :], cwT[:, :], start=True, stop=True
                )
                t = cwrep.tile([128, BS], F32R, tag="cwrep", name=f"cwrep_{k}")
                nc.scalar.copy(t, pbc)  # ACT: keep DVE free for z tiles
                cw_rep.append(t)
            for g in range(4):
                pbc = psum.tile([128, BS], F32, tag="acc", name=f"bcs_{g}")
                nc.tensor.matmul(
                    pbc[:, :], exp2_r[:, g, :, :], cwT[:, :], start=True, stop=True
                )
                t = cwrep.tile([128, BS], F32R, tag="cwstk", name=f"cwstk_{g}")
                nc.scalar.copy(t, pbc)
                cw_stack.append(t)

            b1_sb = persist.tile([K, H], F32R, tag="b1")
            nc.sync.dma_start(out=b1_sb, in_=b1[:, :])
            b2_sb = persist.tile([K, H], F32R, tag="b2")
            nc.sync.dma_start(out=b2_sb, in_=b2[:, :])
            wo_sb = persist.tile([128, OT], F32R, tag="wo")
            nc.sync.dma_start(
                out=wo_sb, in_=Wo.ap().rearrange("(it p) one -> p (it one)", p=128)
            )
            bo_sb = persist.tile([1, 1], F32, tag="bo")
            nc.sync.dma_start(out=bo_sb, in_=bo[:, :])

            # prefetch first W2 k-tiles so L2 starts without DMA latency
            w2_pre = []
            for kt in range(6):
                k, it = kt // OT, kt % OT
                w = w2p.tile([128, H], F32R, tag="w2", name=f"w2pre_{kt}")
                nc.sync.dma_start(out=w, in_=W2[k, bass.ts(it, 128), :])
                w2_pre.append(w)

            # ---- layer 1: h1T[o, b] = relu(W1ext.T @ z1ext) ----
            accs = [psum.tile([128, BS], F32, tag="acc", name=f"acc1_{i}") for i in range(OT)]
            for ot in range(OT):  # bias rows first: shortest dependency chain
                nc.tensor.matmul(
                    accs[ot][:, :],
                    b1_sb[:, bass.ts(ot, 128)],
                    cwT[:, :],
                    start=True,
                    stop=False,
                )
            for k in range(K):  # obs rows: 16 full 128-row slots
                z = zp.tile([128, BS], F32R, tag="z")
                nc.vector.tensor_mul(z, xT0, cw_rep[k])
                w = w1p.tile([128, H], F32R, tag="w1a")
                nc.sync.dma_start(out=w, in_=W1[k, 0:128, :])
                for ot in range(OT):
                    nc.tensor.matmul(
                        accs[ot][:, :],
                        w[:, bass.ts(ot, 128)],
                        z[:, :],
                        start=False,
                        stop=False,
                    )
            for g in range(4):  # action rows: 4 groups of 4 stacked k's
                z = zp.tile([128, BS], F32R, tag="z")
                nc.vector.tensor_mul(z, xT1r4, cw_stack[g])
                w = w1p.tile([128, H], F32R, tag="w1b4")
                for i in range(4):
                    nc.sync.dma_start(
                        out=w[bass.ts(i, ACT), :], in_=W1[4 * g + i, 128:IN1, :]
                    )
                for ot in range(OT):
                    nc.tensor.matmul(
                        accs[ot][:, :],
                        w[:, bass.ts(ot, 128)],
                        z[:, :],
                        start=False,
                        stop=(g == 3),
                    )
            y1 = []
            for ot in range(OT):
                t = ymaj.tile([128, BS], F32R, tag="y1", name=f"y1_{ot}")
                if ot % 2 == 0:
                    nc.scalar.activation(t, accs[ot], mybir.ActivationFunctionType.Relu)
                else:
                    nc.vector.tensor_relu(t, accs[ot])
                y1.append(t)

            # ---- layer 2: h2T[o, b] = relu(W2ext.T @ z2ext) ----
            accs2 = [psum.tile([128, BS], F32, tag="acc", name=f"acc2_{i}") for i in range(OT)]
            for ot in range(OT):  # bias rows first
                nc.tensor.matmul(
                    accs2[ot][:, :],
                    b2_sb[:, bass.ts(ot, 128)],
                    cwT[:, :],
                    start=True,
                    stop=False,
                )
            for kt in range(K * OT):
                k, it = kt // OT, kt % OT
                z = zp.tile([128, BS], F32R, tag="z")
                nc.vector.tensor_mul(z, y1[it], cw_rep[k])
                if kt < 6:
                    w = w2_pre[kt]
                else:
                    w = w2p.tile([128, H], F32R, tag="w2")
                    nc.sync.dma_start(out=w, in_=W2[k, bass.ts(it, 128), :])
                for ot in range(OT):
                    nc.tensor.matmul(
                        accs2[ot][:, :],
                        w[:, bass.ts(ot, 128)],
                        z[:, :],
                        start=False,
                        stop=(kt == K * OT - 1),
                    )
            y2 = []
            for ot in range(OT):
                t = ymaj.tile([128, BS], F32R, tag="y2", name=f"y2_{ot}")
                if ot % 2 == 0:
                    nc.scalar.activation(t, accs2[ot], mybir.ActivationFunctionType.Relu)
                else:
                    nc.vector.tensor_relu(t, accs2[ot])
                y2.append(t)

            # ---- output head: out[b] = sum_o h2T[o, b] * Wo[o] + bo ----
            pso = psum.tile([1, BS], F32, tag="acc")
            for it in range(OT):
                nc.tensor.matmul(
                    pso[:, :],
                    wo_sb[:, it : it + 1],
                    y2[it][:, :],
                    start=(it == 0),
                    stop=(it == OT - 1),
                )
            out_sb = persist.tile([1, BS], F32, tag="out")
            nc.vector.tensor_scalar_add(out_sb, pso, bo_sb)
            nc.sync.dma_start(out=out[:, :], in_=out_sb)

    nc.compile()
    return nc


_NC_CACHE = None


def _get_nc():
    global _NC_CACHE
    if _NC_CACHE is None:
        _NC_CACHE = build_nc()
    return _NC_CACHE


def run(inputs, **spmd_kwargs):
    """Run on 8 cores; returns (full_output [B,1], BassKernelResults)."""
    f32 = lambda a: np.ascontiguousarray(np.asarray(a, dtype=np.float32))
    obs = f32(inputs["obs"])
    act = f32(inputs["actions"])
    cw = f32(inputs["comp_weights"])
    shared = {
        "W1": f32(inputs["W1"]),
        "b1": f32(inputs["b1"]),
        "W2": f32(inputs["W2"]),
        "b2": f32(inputs["b2"]),
        "Wo": f32(inputs["Wo"]),
        "bo": f32(inputs["bo"]).reshape(1, 1),
    }
    in_maps = []
    for c in range(N_CORES):
        s = slice(c * BS, (c + 1) * BS)
        in_maps.append(
            {
                "obs": np.ascontiguousarray(obs[s]),
                "actions": np.ascontiguousarray(act[s]),
                "comp_weights": np.ascontiguousarray(cw[s]),
                **shared,
            }
        )
    res = run_bass_kernel_spmd(
        _get_nc(), in_maps, core_ids=list(range(N_CORES)), **spmd_kwargs
    )
    full = np.concatenate(
        [res.results[c]["out"].reshape(BS, 1) for c in range(N_CORES)], axis=0
    )
    return full, res


def kernel(**inputs) -> np.ndarray:
    return run(inputs)[0]


# revision 14
# speedup vs baseline: 1.0148x; 1.0148x over previous
"""Trainium2 Bass kernel for the CompositionalCritic (nn_CompositionalCritic_18116172054929).

Math (per batch row b):
    x = concat(obs, act)                      # [160]
    h1 = relu(sum_k cw[k] * (x @ W1[k] + b1[k]))   # [1024]
    h2 = relu(sum_k cw[k] * (h1 @ W2[k] + b2[k]))  # [1024]
    out = h2 @ Wo + bo                        # [1]

Key transformation: the soft composition is linear, so
    sum_k cw[k] * (x @ W1[k]) = z @ W1_flat,   z[(k,i)] = cw[k] * x[i]
and the bias term sum_k cw[k]*b1[k] is 16 extra contraction rows with
activations = cw. Each layer becomes ONE dense matmul over an extended
contraction dim; no [B, K, H] intermediate is ever materialized.

Sharding: data-parallel over batch: 8 cores x 512 rows, weights replicated.
All matmuls run in float32r (fp32 storage, near-fp32 accuracy, bf16-class
PE throughput). Activations live feature-major [feat, b] on-chip so the
contraction dim is on partitions for every matmul.
"""

import numpy as np

import concourse.bass as bass
import concourse.mybir as mybir
import concourse.tile as tile
from concourse import bacc
from concourse.bass_utils import run_bass_kernel_spmd
from concourse.masks import make_identity

N_CORES = 8
B, OBS, ACT, K, H = 4096, 128, 32, 16, 1024
IN1 = OBS + ACT  # 160
BS = B // N_CORES  # 512 batch rows per core
NBT = BS // 128  # 4 batch tiles of 128
OT = H // 128  # 8 output tiles per layer
F32 = mybir.dt.float32
F32R = mybir.dt.float32r


def build_nc():
    nc = bacc.Bacc(
        "TRN2",
        target_bir_lowering=False,
        debug=False,
        enable_asserts=False,
        num_devices=N_CORES,
    )

    obs = nc.dram_tensor("obs", [BS, OBS], F32, kind="ExternalInput")
    act = nc.dram_tensor("actions", [BS, ACT], F32, kind="ExternalInput")
    cw = nc.dram_tensor("comp_weights", [BS, K], F32, kind="ExternalInput")
    W1 = nc.dram_tensor("W1", [K, IN1, H], F32R, kind="ExternalInput")
    b1 = nc.dram_tensor("b1", [K, H], F32R, kind="ExternalInput")
    W2 = nc.dram_tensor("W2", [K, H, H], F32R, kind="ExternalInput")
    b2 = nc.dram_tensor("b2", [K, H], F32R, kind="ExternalInput")
    Wo = nc.dram_tensor("Wo", [H, 1], F32R, kind="ExternalInput")
    bo = nc.dram_tensor("bo", [1, 1], F32, kind="ExternalInput")
    out = nc.dram_tensor("out", [1, BS], F32, kind="ExternalOutput")

    with tile.TileContext(nc) as tc:
        with (
            tc.tile_pool(name="persist", bufs=1) as persist,
            tc.tile_pool(name="ld", bufs=3) as ld,
            tc.tile_pool(name="w1p", bufs=3) as w1p,
            tc.tile_pool(name="w2p", bufs=6) as w2p,
            tc.tile_pool(name="zp", bufs=6) as zp,
            tc.tile_pool(name="cwrep", bufs=K) as cwrep,
            tc.tile_pool(name="ymaj", bufs=OT) as ymaj,
            tc.tile_pool(name="psum", bufs=8, space="PSUM") as psum,
        ):
            # ---- phase 0: transpose inputs to feature-major ----
            ident = persist.tile([128, 128], F32, tag="ident")
            make_identity(nc, ident)

            # cw_rep[k][p, b] = cwT[k, b] for all p: PE broadcast via the
            # expander E = kron(I_K, ones(1, 128)); cw_rep[k] = E[:, k].T @ cwT
            # expander[j, k, p] = (j == k): gpsimd affine_select, like make_identity
            expander = persist.tile([K, K, 128], F32, tag="expander")
            nc.gpsimd.memset(expander, 0.0)
            nc.gpsimd.affine_select(
                out=expander,
                in_=expander,
                compare_op=mybir.AluOpType.not_equal,
                fill=1.0,
                base=0,
                pattern=[[-1, K], [0, 128]],
                channel_multiplier=1,
            )
            exp_r = persist.tile([K, K, 128], F32R, tag="exp_r")
            nc.vector.tensor_copy(exp_r, expander)
            # exp2[j, g, ph, pl] = (j == 4g + ph): stacks 4 action-subtiles
            exp2 = persist.tile([K, 4, 4, 32], F32, tag="exp2")
            nc.gpsimd.memset(exp2, 0.0)
            nc.gpsimd.affine_select(
                out=exp2,
                in_=exp2,
                compare_op=mybir.AluOpType.not_equal,
                fill=1.0,
                base=0,
                pattern=[[-4, 4], [-1, 4], [0, 32]],
                channel_multiplier=1,
            )
            exp2_r = persist.tile([K, 4, 4, 32], F32R, tag="exp2_r")
            nc.vector.tensor_copy(exp2_r, exp2)


            xT0 = persist.tile([128, BS], F32R, tag="xT0")  # obsT
            xT1 = persist.tile([ACT, BS], F32R, tag="xT1")  # actionsT
            cwT = persist.tile([K, BS], F32R, tag="cwT")  # cw transposed

            for bt in range(NBT):  # cw first: it gates the broadcast chain
                bsl = bass.ts(bt, 128)
                cwb = ld.tile([128, K], F32, tag="cwb")
                nc.sync.dma_start(out=cwb, in_=cw[bsl, :])
                psc = psum.tile([K, 128], F32, tag="acc", name=f"tpc_{bt}")
                nc.tensor.transpose(psc[:, :], cwb[:, :], ident[:, :])
                nc.vector.tensor_copy(cwT[:, bsl], psc)
            for bt in range(NBT):
                bsl = bass.ts(bt, 128)
                ob = ld.tile([128, OBS], F32, tag="ob")
                nc.sync.dma_start(out=ob, in_=obs[bsl, :])
                pso = psum.tile([OBS, 128], F32, tag="acc", name=f"tpo_{bt}")
                nc.tensor.transpose(pso[:, :], ob[:, :], ident[:, :])
                nc.vector.tensor_copy(xT0[:, bsl], pso)

                ac = ld.tile([128, ACT], F32, tag="ac")
                nc.sync.dma_start(out=ac, in_=act[bsl, :])
                psa_t = psum.tile([ACT, 128], F32, tag="acc", name=f"tpa_{bt}")
                nc.tensor.transpose(psa_t[:, :], ac[:, :], ident[:, :])
                nc.vector.tensor_copy(xT1[:, bsl], psa_t)

            # replicate actionsT 4x vertically for the stacked L1 matmuls
            xT1r4 = persist.tile([128, BS], F32R, tag="xT1r4")
            for i in range(4):
                nc.sync.dma_start(out=xT1r4[bass.ts(i, ACT), :], in_=xT1[:, :])

            cw_rep = []
            cw_stack = []
            for k in range(K):
                pbc = psum.tile([128, BS], F32, tag="acc", name=f"bc_{k}")
                nc.tensor.matmul(
                    pbc[:, :], exp_r[:, k, :], cwT[:, :], start=True, stop=True
                )
                t = cwrep.tile([128, BS], F32R, tag="cwrep", name=f"cwrep_{k}")
                nc.scalar.copy(t, pbc)  # ACT: keep DVE free for z tiles
                cw_rep.append(t)
            for g in range(4):
                pbc = psum.tile([128, BS], F32, tag="acc", name=f"bcs_{g}")
                nc.tensor.matmul(
                    pbc[:, :], exp2_r[:, g, :, :], cwT[:, :], start=True, stop=True
                )
                t = cwrep.tile([128, BS], F32R, tag="cwstk", name=f"cwstk_{g}")
                nc.scalar.copy(t, pbc)
                cw_stack.append(t)

            b1_sb = persist.tile([K, H], F32R, tag="b1")
            nc.sync.dma_start(out=b1_sb, in_=b1[:, :])
            b2_sb = persist.tile([K, H], F32R, tag="b2")
            nc.sync.dma_start(out=b2_sb, in_=b2[:, :])
            wo_sb = persist.tile([128, OT], F32R, tag="wo")
            nc.sync.dma_start(
                out=wo_sb, in_=Wo.ap().rearrange("(it p) one -> p (it one)", p=128)
            )
            bo_sb = persist.tile([1, 1], F32, tag="bo")
            nc.sync.dma_start(out=bo_sb, in_=bo[:, :])

            # prefetch first W2 k-tiles so L2 starts without DMA latency
            w2_pre = []
            for kt in range(6):
                k, it = kt // OT, kt % OT
                w = w2p.tile([128, H], F32R, tag="w2", name=f"w2pre_{kt}")
                nc.sync.dma_start(out=w, in_=W2[k, bass.ts(it, 128), :])
                w2_pre.append(w)

            # ---- layer 1: h1T[o, b] = relu(W1ext.T @ z1ext) ----
            accs = [psum.tile([128, BS], F32, tag="acc", name=f"acc1_{i}") for i in range(OT)]
            for ot in range(OT):  # bias rows first: shortest dependency chain
                nc.tensor.matmul(
                    accs[ot][:, :],
                    b1_sb[:, bass.ts(ot, 128)],
                    cwT[:, :],
                    start=True,
                    stop=False,
                )
            for k in range(K):  # obs rows: 16 full 128-row slots
                z = zp.tile([128, BS], F32R, tag="z")
                nc.vector.tensor_mul(z, xT0, cw_rep[k])
                w = w1p.tile([128, H], F32R, tag="w1a")
                nc.sync.dma_start(out=w, in_=W1[k, 0:128, :])
                for ot in range(OT):
                    nc.tensor.matmul(
                        accs[ot][:, :],
                        w[:, bass.ts(ot, 128)],
                        z[:, :],
                        start=False,
                        stop=False,
                    )
            for g in range(4):  # action rows: 4 groups of 4 stacked k's
                z = zp.tile([128, BS], F32R, tag="z")
                nc.vector.tensor_mul(z, xT1r4, cw_stack[g])
                w = w1p.tile([128, H], F32R, tag="w1b4")
                for i in range(4):
                    nc.sync.dma_start(
                        out=w[bass.ts(i, ACT), :], in_=W1[4 * g + i, 128:IN1, :]
                    )
                for ot in range(OT):
                    nc.tensor.matmul(
                        accs[ot][:, :],
                        w[:, bass.ts(ot, 128)],
                        z[:, :],
                        start=False,
                        stop=(g == 3),
                    )
            y1 = []
            for ot in range(OT):
                t = ymaj.tile([128, BS], F32R, tag="y1", name=f"y1_{ot}")
                nc.scalar.activation(t, accs[ot], mybir.ActivationFunctionType.Relu)
                y1.append(t)

            # ---- layer 2: h2T[o, b] = relu(W2ext.T @ z2ext) ----
            accs2 = [psum.tile([128, BS], F32, tag="acc", name=f"acc2_{i}") for i in range(OT)]
            for ot in range(OT):  # bias rows first
                nc.tensor.matmul(
                    accs2[ot][:, :],
                    b2_sb[:, bass.ts(ot, 128)],
                    cwT[:, :],
                    start=True,
                    stop=False,
                )
            for kt in range(K * OT):
                k, it = kt // OT, kt % OT
                z = zp.tile([128, BS], F32R, tag="z")
                nc.vector.tensor_mul(z, y1[it], cw_rep[k])
                if kt < 6:
                    w = w2_pre[kt]
                else:
                    w = w2p.tile([128, H], F32R, tag="w2")
                    nc.sync.dma_start(out=w, in_=W2[k, bass.ts(it, 128), :])
                for ot in range(OT):
                    nc.tensor.matmul(
                        accs2[ot][:, :],
                        w[:, bass.ts(ot, 128)],
                        z[:, :],
                        start=False,
                        stop=(kt == K * OT - 1),
                    )
            y2 = []
            for ot in range(OT):
                t = ymaj.tile([128, BS], F32R, tag="y2", name=f"y2_{ot}")
                nc.scalar.activation(t, accs2[ot], mybir.ActivationFunctionType.Relu)
                y2.append(t)

            # ---- output head: out[b] = sum_o h2T[o, b] * Wo[o] + bo ----
            pso = psum.tile([1, BS], F32, tag="acc")
            for it in range(OT):
                nc.tensor.matmul(
                    pso[:, :],
                    wo_sb[:, it : it + 1],
                    y2[it][:, :],
                    start=(it == 0),
                    stop=(it == OT - 1),
                )
            out_sb = persist.tile([1, BS], F32, tag="out")
            nc.vector.tensor_scalar_add(out_sb, pso, bo_sb)
            nc.sync.dma_start(out=out[:, :], in_=out_sb)

    nc.compile()
    return nc


_NC_CACHE = None


def _get_nc():
    global _NC_CACHE
    if _NC_CACHE is None:
        _NC_CACHE = build_nc()
    return _NC_CACHE


def run(inputs, **spmd_kwargs):
    """Run on 8 cores; returns (full_output [B,1], BassKernelResults)."""
    f32 = lambda a: np.ascontiguousarray(np.asarray(a, dtype=np.float32))
    obs = f32(inputs["obs"])
    act = f32(inputs["actions"])
    cw = f32(inputs["comp_weights"])
    shared = {
        "W1": f32(inputs["W1"]),
        "b1": f32(inputs["b1"]),
        "W2": f32(inputs["W2"]),
        "b2": f32(inputs["b2"]),
        "Wo": f32(inputs["Wo"]),
        "bo": f32(inputs["bo"]).reshape(1, 1),
    }
    in_maps = []
    for c in range(N_CORES):
        s = slice(c * BS, (c + 1) * BS)
        in_maps.append(
            {
                "obs": np.ascontiguousarray(obs[s]),
                "actions": np.ascontiguousarray(act[s]),
                "comp_weights": np.ascontiguousarray(cw[s]),
                **shared,
            }
        )
    res = run_bass_kernel_spmd(
        _get_nc(), in_maps, core_ids=list(range(N_CORES)), **spmd_kwargs
    )
    full = np.concatenate(
        [res.results[c]["out"].reshape(BS, 1) for c in range(N_CORES)], axis=0
    )
    return full, res


def kernel(**inputs) -> np.ndarray:
    return run(inputs)[0]
